# revision 22
# baseline (speedup 1.0000x reference)
"""Multi-head self-attention (B=2, T=2048, D=1024, H=16) on 8 TRN2 NeuronCores.

Sharding: core c -> (b = c // 4, head-group hg = c % 4); each core computes the
full causal attention + partial output projection for its 4 heads of one batch
element.  Host pre-transposes x, pre-slices Wq (scaled by 1/sqrt(Dh)) / Wk /
Wv columns and Wout rows per head group, and sums the 4 bf16 partial
projections per batch element (+ bout) at the end.

Device-side structure (v2): a single software-pipelined region.
  - A-chunk(ts): kT/qT c-major chains (o-contraction) for query/key tile ts
    plus natural-layout V for its 4 t-blocks.  chunk(0) is the prefix;
    chunk(ts+1) is emitted as PE filler inside attention window ts, so the
    QKV projection hides under the softmax's ScalarE time.
  - B(it): causal attention for 512 queries.  Pairs (2 heads row-packed per
    128 partitions) sweep sequentially so their ctx accumulators share 2 PSUM
    banks; scores are double-buffered (2x2 banks); exp on ScalarE only
    (N=1024 per call), triangle masks + all PSUM drains on VectorE.
    Diagonal-block S/exp/AV are column-sliced to skip fully-masked work.
  - softmax denominators ride the AV matmul via ones-columns in V_aug; the
    hl=1 head stores [ones|V] (swapped) so both heads' ctx land pre-packed
    for the output projection.  Reciprocals are computed lane-parallel on the
    replicated sums rows (DVE), partition-swapped with one SBUF->SBUF DMA,
    and multiplied into merged ctx -- no ScalarE, no DRAM round trip.
  - C(it): output projection for 4 t-blocks, interleaved into the last
    (ScalarE-bound) attention window; bf16 partials DMA out per t-block.
"""

import math
from contextlib import ExitStack

import numpy as np
import ml_dtypes

import concourse.bass as bass
import concourse.bacc as bacc_mod
import concourse.mybir as mybir
import concourse.tile as tile

FP32 = mybir.dt.float32
BF16 = mybir.dt.bfloat16
AF = mybir.ActivationFunctionType
ALU = mybir.AluOpType

B, T, D, H = 2, 2048, 1024, 16
Dh = D // H          # 64
NCORES = 8
HPC = 4              # heads per core
NPAIR = HPC // 2     # head pairs (2 heads share a 128-partition block)
IT = T // 512        # 4 query tiles of 512
JB = T // 128        # 16 key blocks of 128
KO = D // 128        # 8 contraction blocks
SCALE = 1.0 / math.sqrt(Dh)
LAG = 2              # AV emission lag (steps) to avoid FIFO head-of-line stalls


def build_program(compile=True, debug=False):
    nc = bacc_mod.Bacc()
    dbg = {}
    if debug:
        dbg["qkT"] = nc.declare_dram_parameter("dbg_qkT", [4, 128, T], FP32,
                                               isOutput=True)
        dbg["vaug"] = nc.declare_dram_parameter("dbg_vaug",
                                                [128, JB * HPC * 128], FP32,
                                                isOutput=True)
        dbg["merged"] = nc.declare_dram_parameter("dbg_merged",
                                                  [2, 128, IT * 512], FP32,
                                                  isOutput=True)
        dbg["rec"] = nc.declare_dram_parameter("dbg_rec", [128, 512], FP32,
                                               isOutput=True)
        dbg["rec2"] = nc.declare_dram_parameter("dbg_rec2", [128, 512], FP32,
                                                isOutput=True)
        dbg["ctx"] = nc.declare_dram_parameter("dbg_ctx", [128, 1024], FP32,
                                               isOutput=True)

    # host layouts:  xTh[p, ts, o, u] = x[o*128+p, ts*512+u]
    xTh = nc.declare_dram_parameter("xTh", [128, IT, KO, 512], BF16,
                                    isOutput=False)
    wk = nc.declare_dram_parameter("wk", [128, KO, 256], BF16, isOutput=False)
    wq = nc.declare_dram_parameter("wq", [128, KO, 256], BF16, isOutput=False)
    wv = nc.declare_dram_parameter("wv", [128, KO, 256], BF16, isOutput=False)
    wout = nc.declare_dram_parameter("wout", [128, 2, D], BF16, isOutput=False)
    # consts: [tri 128 | ones 64 | bcast-swap 128 | zeros 64]
    consts = nc.declare_dram_parameter("consts", [128, 384], BF16,
                                       isOutput=False)
    out = nc.declare_dram_parameter("out", [T, D], BF16, isOutput=True)
    rec_dram = nc.dram_tensor("rec_dram", [IT, NPAIR, 128, 512], FP32)

    with ExitStack() as ctx:
        tc = ctx.enter_context(tile.TileContext(nc))
        persist = ctx.enter_context(tc.tile_pool(name="persist", bufs=1))
        pb = ctx.enter_context(tc.tile_pool(name="pB", bufs=1))
        psB = ctx.enter_context(tc.tile_pool(name="psB", bufs=1, space="PSUM"))
        psF = ctx.enter_context(tc.tile_pool(name="psF", bufs=1, space="PSUM"))

        # ---------------- persistent tiles ----------------
        qkT = {nm: persist.tile([128, T], BF16, name=nm, tag=nm)
               for nm in ("qT0", "qT1", "kT0", "kT1")}
        V_aug = persist.tile([128, JB, HPC, 128], BF16, name="V_aug",
                             tag="V_aug")
        merged = [
            persist.tile([128, IT, 512], BF16, name=f"merged{p}", tag=f"mg{p}")
            for p in range(NPAIR)
        ]
        wout_sb = persist.tile([128, 2, D], BF16, name="wout_sb", tag="wout_sb")
        consts_sb = persist.tile([128, 384], BF16, name="consts_sb",
                                 tag="consts_sb")
        xT_sb = persist.tile([128, IT, KO, 512], BF16, name="xT_sb",
                             tag="xT_sb")
        wk_sb = persist.tile([128, KO, 256], BF16, name="wk_sb", tag="wk_sb")
        wq_sb = persist.tile([128, KO, 256], BF16, name="wq_sb", tag="wq_sb")
        wv_sb = persist.tile([128, KO, 256], BF16, name="wv_sb", tag="wv_sb")
        scratch = persist.tile([1, 8], BF16, name="scratch", tag="scratch")
        tri = consts_sb[:, 0:128]

        # ---------------- input DMAs (all HWDGE) ----------------
        # sync queue: consts + weights + later xT tiles; scalar queue: the
        # per-o xT blocks of tile 0 (paces the prefix chains) then the exp
        # table preload.
        nc.sync.dma_start(consts_sb[:], consts[:])
        for o in range(KO):
            nc.scalar.dma_start(xT_sb[:, 0, o], xTh[:, 0, o])
        nc.scalar.activation(scratch[0:1, 0:1], consts_sb[0:1, 0:1], AF.Exp)
        for h in range(2):
            osl = slice(4 * h, 4 * h + 4)
            nc.sync.dma_start(wk_sb[:, osl], wk[:, osl])
        for h in range(2):
            osl = slice(4 * h, 4 * h + 4)
            nc.sync.dma_start(wv_sb[:, osl], wv[:, osl])
        for h in range(2):
            osl = slice(4 * h, 4 * h + 4)
            nc.sync.dma_start(wq_sb[:, osl], wq[:, osl])
        nc.sync.dma_start(wout_sb[:], wout[:])
        for ts in range(1, IT):
            nc.sync.dma_start(xT_sb[:, ts], xTh[:, ts])

        # V_aug layouts: hl=0 heads [V(64) | one | zeros(63)] (sums0 on
        # psum row 64, lhsT sliced to 65 cols -> cheaper LDWEIGHTS);
        # hl=1 heads [one | zeros(63) | V(64)] (sums1 on row 0, ctx1 on
        # rows 64:128).  One sums replica is enough: the DRAM-bounce read
        # broadcasts it back to 64 partitions.
        V_e = V_aug.rearrange("p j (pr hl) c -> p j pr hl c", pr=NPAIR)
        nc.vector.tensor_copy(
            V_e[:, :, :, 0, 64:128],
            consts_sb[:, None, None, 320:384].to_broadcast(
                (128, JB, NPAIR, 64)),
        )
        nc.vector.tensor_copy(
            V_e[:, :, :, 1, 0:64],
            consts_sb[:, None, None, 320:384].to_broadcast(
                (128, JB, NPAIR, 64)),
        )
        nc.vector.tensor_copy(
            V_e[:, :, :, 0, 64:65],
            consts_sb[:, None, None, 128:129].to_broadcast(
                (128, JB, NPAIR, 1)),
        )
        nc.vector.tensor_copy(
            V_e[:, :, :, 1, 0:1],
            consts_sb[:, None, None, 128:129].to_broadcast(
                (128, JB, NPAIR, 1)),
        )
        # stage slots hold 1.0 in the rows the finishes never write, so the
        # full-tile reciprocal stays garbage-free
        for _ in range(2):
            stage_init = pb.tile([128, 512], FP32, name="stage", tag="stage",
                                 bufs=2)
            nc.gpsimd.memset(stage_init[:], 1.0)

        # ---------------- filler generators ----------------
        # qk chains: (dest, weight sbuf, col offset); kT first (B needs keys
        # before queries of later tiles)
        CHAINS = [("kT0", wk_sb, 0), ("qT0", wq_sb, 0),
                  ("kT1", wk_sb, 128), ("qT1", wq_sb, 128)]

        def gen_chunk(ts, pairs=(0, 1, 2, 3)):
            """kT/qT tile ts + V t-blocks 4ts..4ts+3, o-interleaved per pair
            (qk N=512 streams hide the V matmuls' LDWEIGHTS)."""
            for vj in pairs:
                nm, wsb, c0 = CHAINS[vj]
                tb = 4 * ts + vj
                ps = psF.tile([128, 512], FP32, name="fq", tag="fillQ", bufs=1)
                psv = psF.tile([128, 256], FP32, name="fv", tag="fillV",
                               bufs=1)
                for o in range(KO):
                    nc.tensor.matmul(
                        ps[:],
                        lhsT=wsb[:, o, c0:c0 + 128],
                        rhs=xT_sb[:, ts, o, :],
                        start=(o == 0), stop=(o == KO - 1),
                    )
                    nc.tensor.matmul(
                        psv[:],
                        lhsT=xT_sb[:, ts, o, 128 * vj: 128 * (vj + 1)],
                        rhs=wv_sb[:, o, :],
                        start=(o == 0), stop=(o == KO - 1),
                    )
                    if o % 2 == 1:
                        yield
                nc.vector.tensor_copy(qkT[nm][:, 512 * ts: 512 * (ts + 1)],
                                      ps[:])
                yield
                psv_e = psv.rearrange("p (pr hl d) -> p pr hl d", pr=NPAIR,
                                      hl=2)
                nc.vector.tensor_copy(V_e[:, tb, :, 0, 0:64],
                                      psv_e[:, :, 0, :])
                nc.vector.tensor_copy(V_e[:, tb, :, 1, 64:128],
                                      psv_e[:, :, 1, :])
                yield

        def gen_c(itc, tail=False):
            """output projection for t-blocks 4itc..4itc+3 (bf16 out DMA)."""
            for j in range(4):
                tb = 4 * itc + j
                osb = pb.tile([128, 2, 512], BF16, name="osb", tag="osb",
                              bufs=2)
                for et in range(2):
                    if tail:
                        # attention is done: reuse the scores banks so the
                        # tail pipeline is double-buffered
                        po = psB.tile([128, 2, 512], FP32, name="fq2",
                                      tag="ps_s", bufs=2)[:, 0, :]
                    else:
                        po = psF.tile([128, 512], FP32, name="fq2",
                                      tag="fillQ" if et == 0 else "fillV",
                                      bufs=1)
                    for pair in range(NPAIR):
                        nc.tensor.matmul(
                            po[:],
                            lhsT=merged[pair][:, itc, 128 * j: 128 * (j + 1)],
                            rhs=wout_sb[:, pair, 512 * et: 512 * (et + 1)],
                            start=(pair == 0), stop=(pair == NPAIR - 1),
                        )
                    yield
                    if et == 0:
                        nc.scalar.copy(osb[:, et], po[:])
                    else:
                        nc.vector.tensor_copy(osb[:, et], po[:])
                    yield
                nc.sync.dma_start(out[128 * tb: 128 * (tb + 1), :], osb[:])

        fillers = []

        def emit_fill(n):
            k = 0
            while k < n and fillers:
                try:
                    next(fillers[0])
                    k += 1
                except StopIteration:
                    fillers.pop(0)

        # ---------------- pipelined attention (one global step list) -------
        ps_map = {}
        pT_map = {}
        ctx_map = {}
        pending_norms = []

        def emit_S(key):
            it, pair, jb = key
            q = jb - 4 * it
            c0 = 128 * max(q, 0)
            kT_t = qkT[f"kT{pair}"]
            qT_t = qkT[f"qT{pair}"]
            ps_s = psB.tile([128, 2, 512], FP32, name="ps_s", tag="ps_s",
                            bufs=2)
            jsl = slice(128 * jb, 128 * (jb + 1))
            for hl in range(2):
                rows = slice(64 * hl, 64 * (hl + 1))
                nc.tensor.matmul(
                    ps_s[:, hl, c0:],
                    lhsT=kT_t[rows, jsl],
                    rhs=qT_t[rows, 512 * it + c0: 512 * (it + 1)],
                    start=True, stop=True,
                )
            ps_map[key] = ps_s

        def emit_exp(key):
            it, pair, jb = key
            q = jb - 4 * it
            c0 = 128 * max(q, 0)
            ps_s = ps_map.pop(key)
            pT = pb.tile([128, 2, 512], BF16, name="pT", tag="pT", bufs=3)
            if q < 0:
                nc.scalar.activation(pT[:], ps_s[:], AF.Exp)
            else:
                nc.scalar.activation(pT[:, :, c0:], ps_s[:, :, c0:], AF.Exp)
                nc.vector.tensor_tensor(
                    out=pT[:, :, c0:c0 + 128],
                    in0=pT[:, :, c0:c0 + 128],
                    in1=tri[:, None, :].to_broadcast((128, 2, 128)),
                    op=ALU.mult,
                )
            pT_map[key] = pT

        def emit_AV(key):
            it, pair, jb = key
            njb = 4 * it + 4
            q = jb - 4 * it
            c0 = 128 * max(q, 0)
            if jb == 0:
                ctx_map[(it, pair)] = psB.tile([128, 2, 512], FP32,
                                               name="ctx", tag="ctx", bufs=1)
            ctx_t = ctx_map[(it, pair)]
            pT = pT_map.pop(key)
            for hl in range(2):
                h = 2 * pair + hl
                ncols = 65 if hl == 0 else 128
                nc.tensor.matmul(
                    ctx_t[0:ncols, hl, c0:],
                    lhsT=V_aug[:, jb, h, 0:ncols],
                    rhs=pT[:, hl, c0:],
                    start=(jb == 0), stop=(jb == njb - 1),
                )
            if jb == njb - 1:
                emit_finish(it, pair)

        def emit_finish(it, pair):
            """ctx0 rows 0:64 / sums0 64:128 (hl=0); sums1 0:64 / ctx1
            64:128 (hl=1).  Drain ctx to SBUF fast (frees the PSUM banks for
            the next sweep), reciprocal the staged sums, DRAM-bounce the
            partition swap, and defer the normalize so the bounce latency
            never blocks the DVE queue."""
            flush_norms()
            ctx_t = ctx_map.pop((it, pair))
            if debug and it == 0 and pair == 0:
                csb = pb.tile([128, 2, 512], FP32, name="csb", tag="csb",
                              bufs=1)
                nc.vector.tensor_copy(csb[:], ctx_t[:])
                nc.gpsimd.dma_start(
                    dbg["ctx"][:], csb.rearrange("p a b -> p (a b)"))
            stage = pb.tile([128, 512], FP32, name="stage", tag="stage",
                            bufs=2)
            rec = pb.tile([128, 512], FP32, name="rec", tag="rec", bufs=2)
            ctxu = pb.tile([128, 512], BF16, name="ctxu", tag="ctxu", bufs=2)
            nc.vector.tensor_copy(ctxu[0:64], ctx_t[0:64, 0, :])
            nc.vector.tensor_copy(ctxu[64:128], ctx_t[64:128, 1, :])
            nc.vector.tensor_copy(stage[0:1], ctx_t[0:1, 1, :])
            nc.vector.tensor_copy(stage[64:65], ctx_t[64:65, 0, :])
            nc.vector.reciprocal_approx_fast(rec[:], stage[:])
            if it == IT - 1 and pair == NPAIR - 1:
                # latency-critical last finish: partition-swap the recips with
                # one PE matmul against a swap identity instead of the DRAM
                # bounce, and normalize inline.
                recb = pb.tile([128, 512], BF16, name="recb", tag="recb",
                               bufs=1)
                nc.vector.tensor_copy(recb[:], rec[:])
                ps_sw = psF.tile([128, 512], FP32, name="ps_sw", tag="fillQ",
                                 bufs=1)
                nc.tensor.matmul(ps_sw[:], lhsT=consts_sb[:, 192:320],
                                 rhs=recb[:], start=True, stop=True)
                nc.vector.tensor_tensor(
                    out=merged[pair][:, it], in0=ctxu[:], in1=ps_sw[:],
                    op=ALU.mult,
                )
                return
            rec2 = pb.tile([128, 512], FP32, name="rec2", tag="rec2", bufs=2)
            nc.sync.dma_start(rec_dram[it, pair], rec[:])
            nc.sync.dma_start(
                rec2[0:64],
                rec_dram[it, pair, 64:65, :].to_broadcast((64, 512)))
            nc.sync.dma_start(
                rec2[64:128],
                rec_dram[it, pair, 0:1, :].to_broadcast((64, 512)))
            if debug and it == 0 and pair == 0:
                nc.gpsimd.dma_start(dbg["rec"][:], rec[:])
                nc.gpsimd.dma_start(dbg["rec2"][:], rec2[:])

            def norm():
                nc.vector.tensor_tensor(
                    out=merged[pair][:, it], in0=ctxu[:], in1=rec2[:],
                    op=ALU.mult,
                )
            pending_norms.append(norm)

        def flush_norms():
            while pending_norms:
                pending_norms.pop(0)()

        # prefix: kT0/qT0 (+V0/V1) run before the first S matmul (which
        # would otherwise head-of-line block the PE queue on its own chain's
        # output); the rest of chunk(0) becomes window-0 filler
        fillers.append(gen_chunk(0, (0, 1)))
        emit_fill(10**9)
        fillers.append(gen_chunk(0, (2, 3)))

        all_steps = [(it, pair, jb)
                     for it in range(IT)
                     for pair in range(NPAIR)
                     for jb in range(4 * it + 4)]
        per_window = {}
        for it in range(IT):
            quanta = (24 if it + 1 < IT else 0) + (48 if it == IT - 1 else 0)
            if it == 0:
                quanta += 12
            per_window[it] = max(1, -(-quanta // (2 * (4 * it + 4))))

        cur_it = -1
        c_setup_countdown = None
        emit_S(all_steps[0])
        for k, key in enumerate(all_steps):
            if key[0] != cur_it:
                cur_it = key[0]
                if cur_it + 1 < IT:
                    fillers.append(gen_chunk(cur_it + 1))
                if cur_it == IT - 1:
                    # wait LAG steps so the previous window's lagged AV +
                    # finish are emitted, then flush all merged norms before
                    # any C filler reads them (program-order RAW)
                    c_setup_countdown = LAG
            emit_exp(key)
            if k + 1 < len(all_steps):
                emit_S(all_steps[k + 1])
            if k >= LAG:
                emit_AV(all_steps[k - LAG])
            if c_setup_countdown is not None:
                c_setup_countdown -= 1
                if c_setup_countdown == 0:
                    flush_norms()
                    for itc in range(IT - 1):
                        fillers.append(gen_c(itc))
                    c_setup_countdown = None
            emit_fill(per_window[cur_it])
        for key in all_steps[-LAG:]:
            emit_AV(key)
        emit_fill(10**9)
        flush_norms()

        # ---------------- tail: last output-projection chunk ----------------
        for _ in gen_c(IT - 1, tail=True):
            pass

        if debug:
            for i, nm in enumerate(("qT0", "qT1", "kT0", "kT1")):
                cv = pb.tile([128, T], FP32, name="cv", tag="cv", bufs=1)
                nc.vector.tensor_copy(cv[:], qkT[nm][:])
                nc.gpsimd.dma_start(dbg["qkT"][i], cv[:])
            cv2 = pb.tile([128, JB * HPC * 128], FP32, name="cv2", tag="cv2",
                          bufs=1)
            nc.vector.tensor_copy(
                cv2[:], V_aug.rearrange("p a b c -> p (a b c)"))
            nc.gpsimd.dma_start(dbg["vaug"][:], cv2[:])
            for p in range(NPAIR):
                cv3 = pb.tile([128, IT * 512], FP32, name="cv3", tag="cv3",
                              bufs=1)
                nc.vector.tensor_copy(
                    cv3[:], merged[p].rearrange("p a b -> p (a b)"))
                nc.gpsimd.dma_start(dbg["merged"][p], cv3[:])

    if compile:
        nc.compile()
    return nc


_PROGRAM = None


def _get_program():
    global _PROGRAM
    if _PROGRAM is None:
        _PROGRAM = build_program()
    return _PROGRAM


def _consts():
    c = np.zeros((128, 384), ml_dtypes.bfloat16)
    dj = np.arange(128)[:, None]
    di = np.arange(128)[None, :]
    c[:, 0:128] = (dj <= di).astype(ml_dtypes.bfloat16)   # causal triangle
    c[:, 128:192] = 1.0                                   # ones columns
    # broadcast-swap as matmul lhsT: out[p<64] = in[64], out[p>=64] = in[0]
    swap = np.zeros((128, 128), np.float32)
    swap[64, 0:64] = 1.0
    swap[0, 64:128] = 1.0
    c[:, 192:320] = swap.astype(ml_dtypes.bfloat16)
    # cols 320:384 stay zero (V_aug filler)
    return c


def _wslice(Wqkv, base, c0, scale=1.0):
    w = Wqkv[:, base + c0: base + c0 + HPC * Dh]
    if scale != 1.0:
        w = w * scale
    w = np.ascontiguousarray(
        w.astype(ml_dtypes.bfloat16).reshape(KO, 128, HPC * Dh)
        .transpose(1, 0, 2))
    return w


def make_in_maps(x, Wqkv, Wout):
    in_maps = []
    for core in range(NCORES):
        b, hg = core // (NCORES // B), core % (NCORES // B)
        c0 = hg * HPC * Dh
        csl = slice(c0, c0 + HPC * Dh)
        xTh = np.ascontiguousarray(
            x[b].T.astype(ml_dtypes.bfloat16)          # [D, T]
            .reshape(KO, 128, IT, 512).transpose(1, 2, 0, 3))
        in_maps.append({
            "consts": _consts(),
            "xTh": xTh,
            "wq": _wslice(Wqkv, 0, c0, SCALE),
            "wk": _wslice(Wqkv, D, c0),
            "wv": _wslice(Wqkv, 2 * D, c0),
            "wout": np.ascontiguousarray(
                Wout[csl, :].astype(ml_dtypes.bfloat16)
                .reshape(2, 128, D).transpose(1, 0, 2)),
        })
    return in_maps


def kernel(x, causal_mask, key_padding_mask, Wqkv, bqkv, Wout, bout,
           _trace=False):
    from concourse.bass_utils import run_bass_kernel_spmd

    x = np.asarray(x, dtype=np.float32)
    Wqkv = np.asarray(Wqkv, dtype=np.float32)
    Wout = np.asarray(Wout, dtype=np.float32)
    bqkv = np.asarray(bqkv, dtype=np.float32)
    bout = np.asarray(bout, dtype=np.float32)
    if np.any(np.asarray(key_padding_mask)):
        raise NotImplementedError("key_padding_mask with padded keys")
    if np.any(bqkv):
        raise NotImplementedError("nonzero bqkv")

    nc = _get_program()
    in_maps = make_in_maps(x, Wqkv, Wout)
    res = run_bass_kernel_spmd(nc, in_maps, core_ids=list(range(NCORES)),
                               trace=_trace)
    G = NCORES // B
    outp = np.empty((B, T, D), dtype=np.float32)
    for b in range(B):
        acc = res.results[b * G]["out"].astype(np.float32)
        for hg in range(1, G):
            acc = acc + res.results[b * G + hg]["out"].astype(np.float32)
        outp[b] = acc + bout
    kernel.last_exec_time_ns = res.exec_time_ns
    return outp
